# revision 7
# baseline (speedup 1.0000x reference)
"""BAD-descriptor kernel for Trainium2 (8 NeuronCores).

Layout: "band" layout — partition q in [0,120) owns output rows [4q, 4q+4)
and holds a 43-row x 743-col window of the (edge-padded) integral image in
its free dimension, so every per-pair row/col shift is a free-dim AP offset.
Per pair: 4 fp32 tensor_sub on DVE + 1 activation (scale+bias) on ACT.
Sharding: 32 pairs per core; one SPMD program with 8 partition-id branches
(per-pair AP offsets are compile-time constants).
Clamped edge strips (offsets pushing boxes past the image border) are
recomputed on host (<~5% of output elements).
"""

import numpy as np

H, W = 480, 640
MR = 3
P_TOTAL = 256
N_CORES = 8
PAIRS_PER_CORE = P_TOTAL // N_CORES
B_ROWS = 4                 # output rows per partition
NPART = H // B_ROWS        # 120
ROW_SLOTS = 43             # band rows: [4q-16 .. 4q+26] of I2D
ROW_PAD = 16               # I_pad row = I2D row + 16
COL_SLOTS = 743            # I_pad cols: I2D col + 48
COL_PAD = 48
W_LO = 32                  # W-chain computed over w' in [32, 711)
W_WIDTH = 679


def _integral(xs: np.ndarray) -> np.ndarray:
    """(487, 647) float32 integral image, matching the reference layout."""
    xp = np.pad(xs, MR, mode="edge")
    ii = np.zeros((H + 2 * MR + 1, W + 2 * MR + 1), dtype=np.float32)
    np.cumsum(np.cumsum(xp, axis=0, dtype=np.float32), axis=1,
              dtype=np.float32, out=ii[1:, 1:])
    return ii


def _build_program(off_y1, off_x1, off_y2, off_x2, radii, thresholds,
                   reps=1, gps_w2=False):
    import concourse.tile as tile
    from concourse import bacc, mybir

    DT = mybir.dt.float32
    nc = bacc.Bacc()
    irep_ext = nc.declare_dram_parameter("irep", [NPART, ROW_SLOTS, COL_SLOTS],
                                         DT, isOutput=False)
    out_ext = nc.declare_dram_parameter("out", [PAIRS_PER_CORE, NPART, B_ROWS, W],
                                        DT, isOutput=True)

    with tile.TileContext(nc) as tc:
        import contextlib
        with contextlib.ExitStack() as ctx:
            ipool = ctx.enter_context(tc.tile_pool(name="ipool", bufs=1))
            wpool = ctx.enter_context(tc.tile_pool(name="wpool", bufs=1))
            opool = ctx.enter_context(tc.tile_pool(name="opool", bufs=2))

            ir = ipool.tile([NPART, ROW_SLOTS, COL_SLOTS], DT)
            nc.sync.dma_start(ir[:], irep_ext[:])

            def one_pair(c, k):
                p = c * PAIRS_PER_CORE + k
                oy1 = int(off_y1[p]); ox1 = int(off_x1[p])
                oy2 = int(off_y2[p]); ox2 = int(off_x2[p])
                r = int(radii[p])
                area = float((2 * r + 1) ** 2)
                th = float(thresholds[p])
                dlt = ox2 - ox1
                # row slots (relative to y_local)
                u1a = oy1 + ROW_PAD + MR + r + 1   # oy1 + r + 20
                u1b = oy1 + ROW_PAD + MR - r       # oy1 + 19 - r
                u2a = oy2 + ROW_PAD + MR + r + 1
                u2b = oy2 + ROW_PAD + MR - r
                # final column-diff offsets (in I-col space, rel to x)
                v1a = ox1 + r + 20
                v1b = ox1 + 19 - r
                # W-chain only needs cols [v1b, v1a + W) of I-col space
                wlen = v1a - v1b + W               # 640 + 2r + 1
                base = W_LO + v1b                  # w'-coord of W-chain col 0

                w1 = wpool.tile([NPART, B_ROWS, wlen], DT, tag="w1")
                nc.vector.tensor_sub(
                    w1[:],
                    ir[:, u1a:u1a + B_ROWS, base:base + wlen],
                    ir[:, u1b:u1b + B_ROWS, base:base + wlen])
                w2 = wpool.tile([NPART, B_ROWS, wlen], DT, tag="w2")
                eng2 = nc.gpsimd if gps_w2 else nc.vector
                eng2.tensor_sub(
                    w2[:],
                    ir[:, u2a:u2a + B_ROWS, base + dlt:base + dlt + wlen],
                    ir[:, u2b:u2b + B_ROWS, base + dlt:base + dlt + wlen])
                w3 = wpool.tile([NPART, B_ROWS, wlen], DT, tag="w3")
                nc.vector.tensor_sub(w3[:], w1[:], w2[:])
                w4 = wpool.tile([NPART, B_ROWS, W], DT, tag="w4")
                nc.vector.tensor_sub(w4[:],
                                     w3[:, :, v1a - v1b:v1a - v1b + W],
                                     w3[:, :, 0:W])
                ot = opool.tile([NPART, B_ROWS, W], DT, tag="ot")
                nc.scalar.activation(
                    ot[:], w4[:], mybir.ActivationFunctionType.Copy,
                    bias=-th, scale=1.0 / area)
                nc.sync.dma_start(out_ext[k], ot[:])

            pid = nc.partition_id()
            for c in range(N_CORES):
                with tc.If(pid == c):
                    if reps == 1:
                        for k in range(PAIRS_PER_CORE):
                            one_pair(c, k)
                    else:
                        with tc.For_i(0, reps):
                            for k in range(PAIRS_PER_CORE):
                                one_pair(c, k)
    nc.finalize()
    return nc


def _host_edges(out, I2D, off_y1, off_x1, off_y2, off_x2, radii, thresholds):
    """Recompute (on host, mirroring the reference exactly) every output
    element whose box center got clamped."""
    ally = np.arange(H, dtype=np.float32)
    allx = np.arange(W, dtype=np.float32)

    def box(oy, ox, r, ys, xs):
        cy = (np.clip(ys + oy, 0.0, float(H - 1))).astype(np.int32) + MR
        cx = (np.clip(xs + ox, 0.0, float(W - 1))).astype(np.int32) + MR
        y0 = (cy - r)[:, None]; y1 = (cy + r + 1)[:, None]
        x0 = (cx - r)[None, :]; x1 = (cx + r + 1)[None, :]
        area_sum = (I2D[y1, x1] - I2D[y0, x1] - I2D[y1, x0] + I2D[y0, x0])
        return area_sum / np.float32((2 * r + 1) ** 2)

    for p in range(P_TOTAL):
        oy1 = float(off_y1[p]); ox1 = float(off_x1[p])
        oy2 = float(off_y2[p]); ox2 = float(off_x2[p])
        r = int(radii[p]); th = np.float32(thresholds[p])
        t = int(max(0.0, -oy1, -oy2)); b = int(max(0.0, oy1, oy2))
        l = int(max(0.0, -ox1, -ox2)); rr = int(max(0.0, ox1, ox2))

        def patch(ys, xs):
            out[p, ys[:, None].astype(np.int32), xs[None, :].astype(np.int32)] = (
                box(oy1, ox1, r, ys, xs) - box(oy2, ox2, r, ys, xs) - th)

        if t:
            patch(ally[:t], allx)
        if b:
            patch(ally[H - b:], allx)
        if l:
            patch(ally, allx[:l])
        if rr:
            patch(ally, allx[W - rr:])
    return out


def _run(x, offset_x1, offset_x2, offset_y1, offset_y2, radii, thresholds,
         trace=False, reps=1, gps_w2=False):
    from concourse.bass_utils import run_bass_kernel_spmd

    x = np.asarray(x); radii_np = np.asarray(radii)
    off_x1 = np.asarray(offset_x1); off_x2 = np.asarray(offset_x2)
    off_y1 = np.asarray(offset_y1); off_y2 = np.asarray(offset_y2)
    th_np = np.asarray(thresholds)

    I2D = _integral(np.asarray(x[0, 0], dtype=np.float32))
    I_pad = np.pad(I2D, ((ROW_PAD, ROW_PAD + 32), (COL_PAD, COL_PAD)),
                   mode="edge")
    swv = np.lib.stride_tricks.sliding_window_view(I_pad, ROW_SLOTS, axis=0)
    irep = np.ascontiguousarray(
        swv[0:H:B_ROWS].transpose(0, 2, 1), dtype=np.float32)  # (120,43,743)

    nc = _build_program(off_y1, off_x1, off_y2, off_x2, radii_np, th_np,
                        reps=reps, gps_w2=gps_w2)
    in_maps = [{"irep": irep} for _ in range(N_CORES)]
    bkr = run_bass_kernel_spmd(nc, in_maps, list(range(N_CORES)), trace=trace)
    res = bkr.results

    out = np.concatenate(
        [np.asarray(res[c]["out"]).reshape(PAIRS_PER_CORE, H, W)
         for c in range(N_CORES)], axis=0)
    out = _host_edges(out, I2D, off_y1, off_x1, off_y2, off_x2, radii_np, th_np)
    return out[None].astype(np.float32, copy=False), bkr


def kernel(x, offset_x1, offset_x2, offset_y1, offset_y2, radii, thresholds):
    out, _ = _run(x, offset_x1, offset_x2, offset_y1, offset_y2, radii,
                  thresholds)
    return out
